# revision 31
# baseline (speedup 1.0000x reference)
"""Masked-linear kernel for trn2: out = x @ (mask.T * w) + b.

Full shapes: x (8192, 3072) f32, w (3072, 1536) f32, b (1536,) f32,
mask (1536, 3072) f32 -> out (8192, 1536) f32.

Strategy: 8 NeuronCores, data-parallel on batch (1024 rows per core);
w / mask / b replicated. Each core computes outT (1536, 1024) f32 =
(w*maskT).T @ x_shard.T + b in bf16 on TensorE with full-K PSUM
accumulation.

The mask is block-structured on a 512x512 grid; blocks are classified
at runtime on the host as all-zero ('z') / all-one ('o') / mixed
('m'). 'z' blocks contribute nothing so their matmuls are skipped
(576 -> 448 PE instructions for the reference mask; the kernel is
TensorE-bound at ~95 us); 'o' blocks skip the mask load and VectorE
premultiply. A module is compiled per observed pattern, so arbitrary
masks still give correct results via the all-'m' path.

DMA trigger issue serializes at ~0.6 us per dma_start on a HWDGE
queue, so transfers are few and large: host packs x and the per-ub
weight(+mask) streams into fully contiguous (128, C) dram buffers in
exact consumption order. x + output triggers go on the sync queue,
weight-stream triggers on the scalar queue. Mixed-block raw streams
rotate through 3 SBUF slots (DVE extracts w*m products to their own
tiles); 'o' streams are read by the PE directly.

Schedule per core: phase A runs the unit-block with the largest K-set
k-major across its 4 unit-chunks (8 PSUM banks) so x streaming
overlaps the PE from the first chunk; its last 8 k-chunks run u-major
to stagger chain endings. Phase B runs the remaining unit-chunks
u-major; PSUM slots recycle in exactly drain-completion order and
drains (+bias, on VectorE) overlap compute, so the PE never waits.
Output tiles DMA out as soon as each chain drains.
"""

import os
import sys

import numpy as np
import ml_dtypes

for _p in ("/opt/trn_rl_repo",):
    if os.path.isdir(_p) and _p not in sys.path:
        sys.path.append(_p)

import concourse.bass as bass  # noqa: E402
import concourse.mybir as mybir  # noqa: E402
import concourse.tile as tile  # noqa: E402
from concourse import bacc  # noqa: E402
from concourse.bass_utils import run_bass_kernel_spmd  # noqa: E402

BF16 = ml_dtypes.bfloat16

BATCH, IN_DIM, UNITS = 8192, 3072, 1536
N_CORES = 8
BC = BATCH // N_CORES  # 1024 batch rows per core
P = 128
KC = IN_DIM // P  # 24 k-chunks
UC = UNITS // P  # 12 u-chunks
BT = 512  # matmul moving free dim (one PSUM bank of f32)
NB = BC // BT  # 2
BLK = 512  # mask classification block edge
UBS = UNITS // BLK  # 3 unit blocks
KBS = IN_DIM // BLK  # 6 input blocks
KPB = BLK // P  # 4 k-chunks per input block
UPB = BLK // P  # 4 u-chunks per unit block
_MODULES = {}


def _classify(mask):
    """Classify each 512x512 block of mask: 'z' all-zero, 'o' all-one,
    'm' anything else. Correct for arbitrary masks (worst case all-'m')."""
    pat = []
    for ub in range(UBS):
        row = []
        for kb in range(KBS):
            blk = mask[ub * BLK : (ub + 1) * BLK, kb * BLK : (kb + 1) * BLK]
            mx = blk.max()
            if mx == 0.0:
                row.append("z")
            elif blk.min() == 1.0 and mx == 1.0:
                row.append("o")
            else:
                row.append("m")
        if all(c == "z" for c in row):
            row[0] = "m"  # keep one accumulation chain alive for this row
        pat.append(tuple(row))
    return tuple(pat)


def _layout(pat, ub):
    """Stream layout for unit-block ub: list of (kb, kind, col_off, width)
    in ascending-kb (= consumption) order. 'o': 4x512 cols of w.
    'm': 4x(512 w + 512 m) cols."""
    ent = []
    off = 0
    for kb in range(KBS):
        cls = pat[ub][kb]
        if cls == "z":
            continue
        wdt = 4 * 512 if cls == "o" else 4 * 1024
        ent.append((kb, cls, off, wdt))
        off += wdt
    return ent, off


def _ub_order(pat):
    ksets = []
    for ub in range(UBS):
        ks = [
            kb * KPB + i
            for kb in range(KBS)
            if pat[ub][kb] != "z"
            for i in range(KPB)
        ]
        ksets.append(ks)
    order = sorted(range(UBS), key=lambda ub: -len(ksets[ub]))
    return order, ksets


def _build_module(pat):
    nc = bacc.Bacc("TRN2", target_bir_lowering=False, debug=False)

    ub_order, ksets = _ub_order(pat)
    ub_A = ub_order[0]
    ksA = ksets[ub_A]
    layouts = {ub: _layout(pat, ub) for ub in range(UBS)}

    xp_d = nc.dram_tensor(
        "xp", (P, KC * BC), mybir.dt.bfloat16, kind="ExternalInput"
    )  # packed xT: col k*1024+b = x[b, k*128+p]
    st_d = [
        nc.dram_tensor(
            f"s{ub}", (P, max(layouts[ub][1], 512)), mybir.dt.bfloat16,
            kind="ExternalInput",
        )
        for ub in range(UBS)
    ]
    bp = nc.dram_tensor("bp", (P, UC), mybir.dt.float32, kind="ExternalInput")
    outT = nc.dram_tensor("outT", (UNITS, BC), mybir.dt.float32, kind="ExternalOutput")
    o3 = outT.ap().rearrange("(u p) b -> u p b", p=P)  # [12, 128, 1024]

    with tile.TileContext(nc) as tc:
        with (
            tc.tile_pool(name="cpool", bufs=1) as cpool,
            tc.tile_pool(name="xpool", bufs=1) as xpool,
            tc.tile_pool(name="opool", bufs=1) as opool,
            tc.tile_pool(name="mspool", bufs=8) as mspool,
            tc.tile_pool(name="prpool", bufs=1) as prpool,
            tc.tile_pool(name="otpool", bufs=8) as otpool,
            tc.tile_pool(name="pspool", bufs=8, space="PSUM") as pspool,
        ):
            # ---- weight/mask streams (scalar HWDGE queue) ----
            otiles = {}  # (ub, kb) -> [128, 2048] tile of w (all-one blocks)
            prods = {}  # (ub, k) -> [128, 512] masked-weight tile
            nstream = [0]

            def emit_stream(ub, eng):
                ents, _ = layouts[ub]
                for kb, cls, off, wdt in ents:
                    if cls == "o":
                        t = otiles[(ub, kb)] = opool.tile(
                            [P, wdt], mybir.dt.bfloat16,
                            name=f"os{ub}_{kb}", tag=f"os{ub}_{kb}",
                        )
                        eng.dma_start(t[:], st_d[ub].ap()[:, off : off + wdt])
                        nstream[0] += KPB
                    else:
                        # one tile per 2 k-chunks: [w_k|m_k|w_k1|m_k1] --
                        # halves the serialized dma_start trigger count
                        kstep = 2
                        for ki in range(0, KPB, kstep):
                            k = kb * KPB + ki
                            ko = off + ki * 1024
                            t = mspool.tile(
                                [P, kstep * 1024], mybir.dt.bfloat16,
                                name=f"ms{ub}_{k}", tag="ms",
                            )
                            nsp = 2 if nstream[0] < 4 else 1
                            step = kstep * 1024 // nsp
                            for s in range(nsp):
                                eng.dma_start(
                                    t[:, s * step : (s + 1) * step],
                                    st_d[ub].ap()[:, ko + s * step : ko + (s + 1) * step],
                                )
                            for kk in range(kstep):
                                pr = prods[(ub, k + kk)] = prpool.tile(
                                    [P, BLK], mybir.dt.bfloat16,
                                    name=f"pr{ub}_{k + kk}", tag=f"pr{ub}_{k + kk}",
                                )
                                nc.vector.tensor_mul(
                                    pr[:],
                                    t[:, kk * 1024 : kk * 1024 + 512],
                                    t[:, kk * 1024 + 512 : (kk + 1) * 1024],
                                )
                            nstream[0] += kstep

            def wslice(u, k):
                ub, j = divmod(u, UPB)
                kb, ki = divmod(k, KPB)
                if pat[ub][kb] == "o":
                    return otiles[(ub, kb)][:, ki * 512 + j * P : ki * 512 + (j + 1) * P]
                return prods[(ub, k)][:, j * P : (j + 1) * P]

            # ---- x loads (sync HWDGE queue), consumption order ----
            xt = []
            for k in range(KC):
                t = xpool.tile([P, BC], mybir.dt.bfloat16, name=f"x{k}", tag=f"x{k}")
                nsp = 2 if k < 2 else 1
                step = BC // nsp
                for s in range(nsp):
                    nc.sync.dma_start(
                        t[:, s * step : (s + 1) * step],
                        xp_d.ap()[:, k * BC + s * step : k * BC + (s + 1) * step],
                    )
                xt.append(t)

            btile = cpool.tile([P, UC], mybir.dt.float32, name="btile", tag="btile")
            nc.sync.dma_start(btile[:], bp.ap())

            def xslice(k, b):
                return xt[k][:, b * BT : (b + 1) * BT]

            # stream emission: phase A's on the scalar queue (issues from
            # t~7), the other unit-blocks' on the sync queue right after x
            # (issue ~24+, arrive ~40) so their DVE premultiplies -- and
            # therefore phase A's drains behind them in the in-order DVE
            # queue -- are never late; late drains stall every matmul
            # emitted after them
            emit_stream(ub_A, nc.scalar)
            if len(ub_order) > 1:
                emit_stream(ub_order[1], nc.scalar)

            def chain_mm(u, b, ps, k, ks):
                nc.tensor.matmul(
                    ps[:],
                    wslice(u, k),
                    xslice(k, b),
                    start=(k == ks[0]),
                    stop=(k == ks[-1]),
                )

            ndrained = [0]

            def drain(u, b, ps):
                ot = otpool.tile([P, BT], mybir.dt.float32, name=f"ot{u}_{b}", tag="ot")
                if ndrained[0] < 2 * UC - 2:
                    # ScalarE (idle once stream triggers are out); keeps
                    # drains off the in-order DVE mul queue
                    nc.scalar.add(ot[:], ps[:], btile[:, u : u + 1])
                elif ndrained[0] == 2 * UC - 1:
                    # final drain: two half-drains so both half-stores
                    # launch on parallel queues immediately
                    H = BT // 2
                    dst = o3[u][:, b * BT : (b + 1) * BT]
                    for h in range(2):
                        nc.vector.tensor_add(
                            ot[:, h * H : (h + 1) * H],
                            ps[:, h * H : (h + 1) * H],
                            btile[:, u : u + 1].to_broadcast((P, H)),
                        )
                        q = H // 2
                        for s in range(2):
                            eng = nc.sync if s == 0 else nc.scalar
                            lo = h * H + s * q
                            eng.dma_start(dst[:, lo : lo + q], ot[:, lo : lo + q])
                    ndrained[0] += 1
                    return
                else:
                    nc.vector.tensor_add(
                        ot[:], ps[:], btile[:, u : u + 1].to_broadcast((P, BT))
                    )
                ndrained[0] += 1
                dst = o3[u][:, b * BT : (b + 1) * BT]
                # progressively finer splits near the end, alternating HWDGE
                # queues, so the final transfers don't serialize the tail
                if ndrained[0] >= 2 * UC - 1:
                    nsp = 4
                elif ndrained[0] >= 2 * UC - 3:
                    nsp = 2
                else:
                    nsp = 1
                step = BT // nsp
                for s in range(nsp):
                    eng = nc.scalar if (nsp > 1 and s % 2) else nc.sync
                    eng.dma_start(
                        dst[:, s * step : (s + 1) * step],
                        ot[:, s * step : (s + 1) * step],
                    )

            # ---- phase A: k-major over 4 u-chunks (8 banks), u-major tail ----
            uA = [ub_A * UPB + j for j in range(UPB)]
            psA = {}
            for u in uA:
                for b in range(NB):
                    psA[(u, b)] = pspool.tile(
                        [P, BT], mybir.dt.float32, name=f"ps{u}_{b}", tag="ps"
                    )
            split = max(0, len(ksA) - 8)
            for k in ksA[:split]:
                for u in uA:
                    for b in range(NB):
                        chain_mm(u, b, psA[(u, b)], k, ksA)
            for u in uA:
                for k in ksA[split:]:
                    for b in range(NB):
                        chain_mm(u, b, psA[(u, b)], k, ksA)
                for b in range(NB):
                    drain(u, b, psA[(u, b)])

            if len(ub_order) > 2:
                emit_stream(ub_order[2], nc.scalar)

            # ---- phase B: remaining unit-chunks u-major ----
            for ub in ub_order[1:]:
                ks = ksets[ub]
                for j in range(UPB):
                    u = ub * UPB + j
                    pss = [
                        pspool.tile(
                            [P, BT], mybir.dt.float32, name=f"ps{u}_{b}", tag="ps"
                        )
                        for b in range(NB)
                    ]
                    if ub == ub_order[-1] and j == UPB - 1:
                        # b-serial: only the final b-tile's drain + store
                        # remains after the very last matmul
                        for b in range(NB):
                            for k in ks:
                                chain_mm(u, b, pss[b], k, ks)
                            drain(u, b, pss[b])
                    else:
                        for k in ks:
                            for b in range(NB):
                                chain_mm(u, b, pss[b], k, ks)
                        for b in range(NB):
                            drain(u, b, pss[b])

    nc.compile()
    return nc


def get_module(pat):
    if pat not in _MODULES:
        _MODULES[pat] = _build_module(pat)
    return _MODULES[pat]


def make_in_maps(pat, x, w, b, mask):
    x16 = x.astype(BF16)
    w16 = w.astype(BF16)  # (3072, 1536)
    m16 = np.ascontiguousarray(mask.T).astype(BF16)  # (3072, 1536)
    shared = {"bp": np.ascontiguousarray(b.astype(np.float32).reshape(UC, P).T)}
    for ub in range(UBS):
        wk = np.ascontiguousarray(w16[:, ub * BLK : (ub + 1) * BLK]).reshape(
            KC, P, BLK
        )
        mk = np.ascontiguousarray(m16[:, ub * BLK : (ub + 1) * BLK]).reshape(
            KC, P, BLK
        )
        ents, total = _layout(pat, ub)
        stream = np.zeros((P, max(total, 512)), dtype=BF16)
        for kb, cls, off, wdt in ents:
            for ki in range(KPB):
                k = kb * KPB + ki
                if cls == "o":
                    stream[:, off + ki * 512 : off + (ki + 1) * 512] = wk[k]
                else:
                    stream[:, off + ki * 1024 : off + ki * 1024 + 512] = wk[k]
                    stream[:, off + ki * 1024 + 512 : off + (ki + 1) * 1024] = mk[k]
        shared[f"s{ub}"] = stream
    in_maps = []
    for c in range(N_CORES):
        d = dict(shared)
        xc = np.ascontiguousarray(x16[c * BC : (c + 1) * BC].T)  # (3072, 1024)
        d["xp"] = np.ascontiguousarray(
            xc.reshape(KC, P, BC).transpose(1, 0, 2).reshape(P, KC * BC)
        )
        in_maps.append(d)
    return in_maps


def assemble(results):
    out = np.empty((BATCH, UNITS), dtype=np.float32)
    for c in range(N_CORES):
        out[c * BC : (c + 1) * BC, :] = results[c]["outT"].T
    return out


def kernel(x, w, b, mask, _trace=False, _trace_kwargs=None):
    x = np.asarray(x, dtype=np.float32)
    w = np.asarray(w, dtype=np.float32)
    b = np.asarray(b, dtype=np.float32)
    mask = np.asarray(mask, dtype=np.float32)
    pat = _classify(mask)
    nc = get_module(pat)
    in_maps = make_in_maps(pat, x, w, b, mask)
    res = run_bass_kernel_spmd(
        nc,
        in_maps,
        core_ids=list(range(N_CORES)),
        trace=_trace,
        **(_trace_kwargs or {}),
    )
    out = assemble(res.results)
    if _trace:
        return out, res
    return out


# revision 32
# speedup vs baseline: 1.0264x; 1.0264x over previous
"""Masked-linear kernel for trn2: out = x @ (mask.T * w) + b.

Full shapes: x (8192, 3072) f32, w (3072, 1536) f32, b (1536,) f32,
mask (1536, 3072) f32 -> out (8192, 1536) f32.

Strategy: 8 NeuronCores, data-parallel on batch (1024 rows per core);
w / mask / b replicated. Each core computes outT (1536, 1024) f32 =
(w*maskT).T @ x_shard.T + b in bf16 on TensorE with full-K PSUM
accumulation.

The mask is block-structured on a 512x512 grid; blocks are classified
at runtime on the host as all-zero ('z') / all-one ('o') / mixed
('m'). 'z' blocks contribute nothing so their matmuls are skipped
(576 -> 448 PE instructions for the reference mask; the kernel is
TensorE-bound at ~95 us); 'o' blocks skip the mask load and VectorE
premultiply. A module is compiled per observed pattern, so arbitrary
masks still give correct results via the all-'m' path.

DMA trigger issue serializes at ~0.6 us per dma_start on a HWDGE
queue, so transfers are few and large: host packs x and the per-ub
weight(+mask) streams into fully contiguous (128, C) dram buffers in
exact consumption order. x + output triggers go on the sync queue,
weight-stream triggers on the scalar queue. Mixed-block raw streams
rotate through 3 SBUF slots (DVE extracts w*m products to their own
tiles); 'o' streams are read by the PE directly.

Schedule per core: phase A runs the unit-block with the largest K-set
k-major across its 4 unit-chunks (8 PSUM banks) so x streaming
overlaps the PE from the first chunk; its last 8 k-chunks run u-major
to stagger chain endings. Phase B runs the remaining unit-chunks
u-major; PSUM slots recycle in exactly drain-completion order and
drains (+bias, on VectorE) overlap compute, so the PE never waits.
Output tiles DMA out as soon as each chain drains.
"""

import os
import sys

import numpy as np
import ml_dtypes

for _p in ("/opt/trn_rl_repo",):
    if os.path.isdir(_p) and _p not in sys.path:
        sys.path.append(_p)

import concourse.bass as bass  # noqa: E402
import concourse.mybir as mybir  # noqa: E402
import concourse.tile as tile  # noqa: E402
from concourse import bacc  # noqa: E402
from concourse.bass_utils import run_bass_kernel_spmd  # noqa: E402

BF16 = ml_dtypes.bfloat16

BATCH, IN_DIM, UNITS = 8192, 3072, 1536
N_CORES = 8
BC = BATCH // N_CORES  # 1024 batch rows per core
P = 128
KC = IN_DIM // P  # 24 k-chunks
UC = UNITS // P  # 12 u-chunks
BT = 512  # matmul moving free dim (one PSUM bank of f32)
NB = BC // BT  # 2
BLK = 512  # mask classification block edge
UBS = UNITS // BLK  # 3 unit blocks
KBS = IN_DIM // BLK  # 6 input blocks
KPB = BLK // P  # 4 k-chunks per input block
UPB = BLK // P  # 4 u-chunks per unit block
_MODULES = {}


def _classify(mask):
    """Classify each 512x512 block of mask: 'z' all-zero, 'o' all-one,
    'm' anything else. Correct for arbitrary masks (worst case all-'m')."""
    pat = []
    for ub in range(UBS):
        row = []
        for kb in range(KBS):
            blk = mask[ub * BLK : (ub + 1) * BLK, kb * BLK : (kb + 1) * BLK]
            mx = blk.max()
            if mx == 0.0:
                row.append("z")
            elif blk.min() == 1.0 and mx == 1.0:
                row.append("o")
            else:
                row.append("m")
        if all(c == "z" for c in row):
            row[0] = "m"  # keep one accumulation chain alive for this row
        pat.append(tuple(row))
    return tuple(pat)


def _layout(pat, ub):
    """Stream layout for unit-block ub: list of (kb, kind, col_off, width)
    in ascending-kb (= consumption) order. 'o': 4x512 cols of w.
    'm': 4x(512 w + 512 m) cols."""
    ent = []
    off = 0
    for kb in range(KBS):
        cls = pat[ub][kb]
        if cls == "z":
            continue
        wdt = 4 * 512 if cls == "o" else 4 * 1024
        ent.append((kb, cls, off, wdt))
        off += wdt
    return ent, off


def _ub_order(pat):
    ksets = []
    for ub in range(UBS):
        ks = [
            kb * KPB + i
            for kb in range(KBS)
            if pat[ub][kb] != "z"
            for i in range(KPB)
        ]
        ksets.append(ks)
    order = sorted(range(UBS), key=lambda ub: -len(ksets[ub]))
    return order, ksets


def _build_module(pat):
    nc = bacc.Bacc("TRN2", target_bir_lowering=False, debug=False)

    ub_order, ksets = _ub_order(pat)
    ub_A = ub_order[0]
    ksA = ksets[ub_A]
    layouts = {ub: _layout(pat, ub) for ub in range(UBS)}

    xp_d = nc.dram_tensor(
        "xp", (P, KC * BC), mybir.dt.bfloat16, kind="ExternalInput"
    )  # packed xT: col k*1024+b = x[b, k*128+p]
    st_d = [
        nc.dram_tensor(
            f"s{ub}", (P, max(layouts[ub][1], 512)), mybir.dt.bfloat16,
            kind="ExternalInput",
        )
        for ub in range(UBS)
    ]
    bp = nc.dram_tensor("bp", (P, UC), mybir.dt.float32, kind="ExternalInput")
    outT = nc.dram_tensor("outT", (UNITS, BC), mybir.dt.float32, kind="ExternalOutput")
    o3 = outT.ap().rearrange("(u p) b -> u p b", p=P)  # [12, 128, 1024]

    with tile.TileContext(nc) as tc:
        with (
            tc.tile_pool(name="cpool", bufs=1) as cpool,
            tc.tile_pool(name="xpool", bufs=1) as xpool,
            tc.tile_pool(name="opool", bufs=1) as opool,
            tc.tile_pool(name="mspool", bufs=8) as mspool,
            tc.tile_pool(name="prpool", bufs=1) as prpool,
            tc.tile_pool(name="otpool", bufs=8) as otpool,
            tc.tile_pool(name="pspool", bufs=8, space="PSUM") as pspool,
        ):
            # ---- weight/mask streams (scalar HWDGE queue) ----
            otiles = {}  # (ub, kb) -> [128, 2048] tile of w (all-one blocks)
            prods = {}  # (ub, k) -> [128, 512] masked-weight tile
            nstream = [0]

            def emit_stream(ub, eng):
                ents, _ = layouts[ub]
                for kb, cls, off, wdt in ents:
                    if cls == "o":
                        t = otiles[(ub, kb)] = opool.tile(
                            [P, wdt], mybir.dt.bfloat16,
                            name=f"os{ub}_{kb}", tag=f"os{ub}_{kb}",
                        )
                        eng.dma_start(t[:], st_d[ub].ap()[:, off : off + wdt])
                        nstream[0] += KPB
                    else:
                        # one tile per 2 k-chunks: [w_k|m_k|w_k1|m_k1] --
                        # halves the serialized dma_start trigger count
                        kstep = 2
                        for ki in range(0, KPB, kstep):
                            k = kb * KPB + ki
                            ko = off + ki * 1024
                            t = mspool.tile(
                                [P, kstep * 1024], mybir.dt.bfloat16,
                                name=f"ms{ub}_{k}", tag="ms",
                            )
                            nsp = 2 if nstream[0] < 4 else 1
                            step = kstep * 1024 // nsp
                            for s in range(nsp):
                                eng.dma_start(
                                    t[:, s * step : (s + 1) * step],
                                    st_d[ub].ap()[:, ko + s * step : ko + (s + 1) * step],
                                )
                            for kk in range(kstep):
                                pr = prods[(ub, k + kk)] = prpool.tile(
                                    [P, BLK], mybir.dt.bfloat16,
                                    name=f"pr{ub}_{k + kk}", tag=f"pr{ub}_{k + kk}",
                                )
                                nc.vector.tensor_mul(
                                    pr[:],
                                    t[:, kk * 1024 : kk * 1024 + 512],
                                    t[:, kk * 1024 + 512 : (kk + 1) * 1024],
                                )
                            nstream[0] += kstep

            def wslice(u, k):
                ub, j = divmod(u, UPB)
                kb, ki = divmod(k, KPB)
                if pat[ub][kb] == "o":
                    return otiles[(ub, kb)][:, ki * 512 + j * P : ki * 512 + (j + 1) * P]
                return prods[(ub, k)][:, j * P : (j + 1) * P]

            # ---- x loads (sync HWDGE queue), consumption order ----
            xt = []
            for k in range(KC):
                t = xpool.tile([P, BC], mybir.dt.bfloat16, name=f"x{k}", tag=f"x{k}")
                nsp = 2 if k < 2 else 1
                step = BC // nsp
                for s in range(nsp):
                    nc.sync.dma_start(
                        t[:, s * step : (s + 1) * step],
                        xp_d.ap()[:, k * BC + s * step : k * BC + (s + 1) * step],
                    )
                xt.append(t)

            btile = cpool.tile([P, UC], mybir.dt.float32, name="btile", tag="btile")
            nc.sync.dma_start(btile[:], bp.ap())

            def xslice(k, b):
                return xt[k][:, b * BT : (b + 1) * BT]

            # stream emission: phase A's on the scalar queue (issues from
            # t~7), the other unit-blocks' on the sync queue right after x
            # (issue ~24+, arrive ~40) so their DVE premultiplies -- and
            # therefore phase A's drains behind them in the in-order DVE
            # queue -- are never late; late drains stall every matmul
            # emitted after them
            emit_stream(ub_A, nc.scalar)
            if len(ub_order) > 1:
                emit_stream(ub_order[1], nc.scalar)

            def chain_mm(u, b, ps, k, ks):
                nc.tensor.matmul(
                    ps[:],
                    wslice(u, k),
                    xslice(k, b),
                    start=(k == ks[0]),
                    stop=(k == ks[-1]),
                )

            ndrained = [0]

            def drain(u, b, ps):
                ot = otpool.tile([P, BT], mybir.dt.float32, name=f"ot{u}_{b}", tag="ot")
                if ndrained[0] < 8:
                    # phase A: ScalarE (idle once stream triggers are out);
                    # keeps drains off the in-order DVE mul queue
                    nc.scalar.add(ot[:], ps[:], btile[:, u : u + 1])
                elif ndrained[0] == 2 * UC - 1:
                    # final drain: two half-drains so both half-stores
                    # launch on parallel queues immediately
                    H = BT // 2
                    dst = o3[u][:, b * BT : (b + 1) * BT]
                    for h in range(2):
                        nc.vector.tensor_add(
                            ot[:, h * H : (h + 1) * H],
                            ps[:, h * H : (h + 1) * H],
                            btile[:, u : u + 1].to_broadcast((P, H)),
                        )
                        q = H // 2
                        for s in range(2):
                            eng = nc.sync if s == 0 else nc.scalar
                            lo = h * H + s * q
                            eng.dma_start(dst[:, lo : lo + q], ot[:, lo : lo + q])
                    ndrained[0] += 1
                    return
                else:
                    nc.vector.tensor_add(
                        ot[:], ps[:], btile[:, u : u + 1].to_broadcast((P, BT))
                    )
                ndrained[0] += 1
                dst = o3[u][:, b * BT : (b + 1) * BT]
                # progressively finer splits near the end, alternating HWDGE
                # queues, so the final transfers don't serialize the tail
                if ndrained[0] >= 2 * UC - 1:
                    nsp = 4
                elif ndrained[0] >= 2 * UC - 3:
                    nsp = 2
                else:
                    nsp = 1
                step = BT // nsp
                for s in range(nsp):
                    eng = nc.scalar if (nsp > 1 and s % 2) else nc.sync
                    eng.dma_start(
                        dst[:, s * step : (s + 1) * step],
                        ot[:, s * step : (s + 1) * step],
                    )

            # ---- phase A: k-major over 4 u-chunks (8 banks), u-major tail ----
            uA = [ub_A * UPB + j for j in range(UPB)]
            psA = {}
            for u in uA:
                for b in range(NB):
                    psA[(u, b)] = pspool.tile(
                        [P, BT], mybir.dt.float32, name=f"ps{u}_{b}", tag="ps"
                    )
            split = max(0, len(ksA) - 8)
            for k in ksA[:split]:
                for u in uA:
                    for b in range(NB):
                        chain_mm(u, b, psA[(u, b)], k, ksA)
            for u in uA:
                for k in ksA[split:]:
                    for b in range(NB):
                        chain_mm(u, b, psA[(u, b)], k, ksA)
                for b in range(NB):
                    drain(u, b, psA[(u, b)])

            if len(ub_order) > 2:
                emit_stream(ub_order[2], nc.scalar)

            # ---- phase B: remaining unit-chunks u-major ----
            for ub in ub_order[1:]:
                ks = ksets[ub]
                for j in range(UPB):
                    u = ub * UPB + j
                    pss = [
                        pspool.tile(
                            [P, BT], mybir.dt.float32, name=f"ps{u}_{b}", tag="ps"
                        )
                        for b in range(NB)
                    ]
                    if ub == ub_order[-1] and j == UPB - 1:
                        # b-serial: only the final b-tile's drain + store
                        # remains after the very last matmul
                        for b in range(NB):
                            for k in ks:
                                chain_mm(u, b, pss[b], k, ks)
                            drain(u, b, pss[b])
                    else:
                        for k in ks:
                            for b in range(NB):
                                chain_mm(u, b, pss[b], k, ks)
                        for b in range(NB):
                            drain(u, b, pss[b])

    nc.compile()
    return nc


def get_module(pat):
    if pat not in _MODULES:
        _MODULES[pat] = _build_module(pat)
    return _MODULES[pat]


def make_in_maps(pat, x, w, b, mask):
    x16 = x.astype(BF16)
    w16 = w.astype(BF16)  # (3072, 1536)
    m16 = np.ascontiguousarray(mask.T).astype(BF16)  # (3072, 1536)
    shared = {"bp": np.ascontiguousarray(b.astype(np.float32).reshape(UC, P).T)}
    for ub in range(UBS):
        wk = np.ascontiguousarray(w16[:, ub * BLK : (ub + 1) * BLK]).reshape(
            KC, P, BLK
        )
        mk = np.ascontiguousarray(m16[:, ub * BLK : (ub + 1) * BLK]).reshape(
            KC, P, BLK
        )
        ents, total = _layout(pat, ub)
        stream = np.zeros((P, max(total, 512)), dtype=BF16)
        for kb, cls, off, wdt in ents:
            for ki in range(KPB):
                k = kb * KPB + ki
                if cls == "o":
                    stream[:, off + ki * 512 : off + (ki + 1) * 512] = wk[k]
                else:
                    stream[:, off + ki * 1024 : off + ki * 1024 + 512] = wk[k]
                    stream[:, off + ki * 1024 + 512 : off + (ki + 1) * 1024] = mk[k]
        shared[f"s{ub}"] = stream
    in_maps = []
    for c in range(N_CORES):
        d = dict(shared)
        xc = np.ascontiguousarray(x16[c * BC : (c + 1) * BC].T)  # (3072, 1024)
        d["xp"] = np.ascontiguousarray(
            xc.reshape(KC, P, BC).transpose(1, 0, 2).reshape(P, KC * BC)
        )
        in_maps.append(d)
    return in_maps


def assemble(results):
    out = np.empty((BATCH, UNITS), dtype=np.float32)
    for c in range(N_CORES):
        out[c * BC : (c + 1) * BC, :] = results[c]["outT"].T
    return out


def kernel(x, w, b, mask, _trace=False, _trace_kwargs=None):
    x = np.asarray(x, dtype=np.float32)
    w = np.asarray(w, dtype=np.float32)
    b = np.asarray(b, dtype=np.float32)
    mask = np.asarray(mask, dtype=np.float32)
    pat = _classify(mask)
    nc = get_module(pat)
    in_maps = make_in_maps(pat, x, w, b, mask)
    res = run_bass_kernel_spmd(
        nc,
        in_maps,
        core_ids=list(range(N_CORES)),
        trace=_trace,
        **(_trace_kwargs or {}),
    )
    out = assemble(res.results)
    if _trace:
        return out, res
    return out


# revision 33
# speedup vs baseline: 1.0381x; 1.0114x over previous
"""Masked-linear kernel for trn2: out = x @ (mask.T * w) + b.

Full shapes: x (8192, 3072) f32, w (3072, 1536) f32, b (1536,) f32,
mask (1536, 3072) f32 -> out (8192, 1536) f32.

Strategy: 8 NeuronCores, data-parallel on batch (1024 rows per core);
w / mask / b replicated. Each core computes outT (1536, 1024) f32 =
(w*maskT).T @ x_shard.T + b in bf16 on TensorE with full-K PSUM
accumulation.

The mask is block-structured on a 512x512 grid; blocks are classified
at runtime on the host as all-zero ('z') / all-one ('o') / mixed
('m'). 'z' blocks contribute nothing so their matmuls are skipped
(576 -> 448 PE instructions for the reference mask; the kernel is
TensorE-bound at ~95 us); 'o' blocks skip the mask load and VectorE
premultiply. A module is compiled per observed pattern, so arbitrary
masks still give correct results via the all-'m' path.

DMA trigger issue serializes at ~0.6 us per dma_start on a HWDGE
queue, so transfers are few and large: host packs x and the per-ub
weight(+mask) streams into fully contiguous (128, C) dram buffers in
exact consumption order. x + output triggers go on the sync queue,
weight-stream triggers on the scalar queue. Mixed-block raw streams
rotate through 3 SBUF slots (DVE extracts w*m products to their own
tiles); 'o' streams are read by the PE directly.

Schedule per core: phase A runs the unit-block with the largest K-set
k-major across its 4 unit-chunks (8 PSUM banks) so x streaming
overlaps the PE from the first chunk; its last 8 k-chunks run u-major
to stagger chain endings. Phase B runs the remaining unit-chunks
u-major; PSUM slots recycle in exactly drain-completion order and
drains (+bias, on VectorE) overlap compute, so the PE never waits.
Output tiles DMA out as soon as each chain drains.
"""

import os
import sys

import numpy as np
import ml_dtypes

for _p in ("/opt/trn_rl_repo",):
    if os.path.isdir(_p) and _p not in sys.path:
        sys.path.append(_p)

import concourse.bass as bass  # noqa: E402
import concourse.mybir as mybir  # noqa: E402
import concourse.tile as tile  # noqa: E402
from concourse import bacc  # noqa: E402
from concourse.bass_utils import run_bass_kernel_spmd  # noqa: E402

BF16 = ml_dtypes.bfloat16

BATCH, IN_DIM, UNITS = 8192, 3072, 1536
N_CORES = 8
BC = BATCH // N_CORES  # 1024 batch rows per core
P = 128
KC = IN_DIM // P  # 24 k-chunks
UC = UNITS // P  # 12 u-chunks
BT = 512  # matmul moving free dim (one PSUM bank of f32)
NB = BC // BT  # 2
BLK = 512  # mask classification block edge
UBS = UNITS // BLK  # 3 unit blocks
KBS = IN_DIM // BLK  # 6 input blocks
KPB = BLK // P  # 4 k-chunks per input block
UPB = BLK // P  # 4 u-chunks per unit block
_MODULES = {}


def _classify(mask):
    """Classify each 512x512 block of mask: 'z' all-zero, 'o' all-one,
    'm' anything else. Correct for arbitrary masks (worst case all-'m')."""
    pat = []
    for ub in range(UBS):
        row = []
        for kb in range(KBS):
            blk = mask[ub * BLK : (ub + 1) * BLK, kb * BLK : (kb + 1) * BLK]
            mx = blk.max()
            if mx == 0.0:
                row.append("z")
            elif blk.min() == 1.0 and mx == 1.0:
                row.append("o")
            else:
                row.append("m")
        if all(c == "z" for c in row):
            row[0] = "m"  # keep one accumulation chain alive for this row
        pat.append(tuple(row))
    return tuple(pat)


def _layout(pat, ub):
    """Stream layout for unit-block ub: list of (kb, kind, col_off, width)
    in ascending-kb (= consumption) order. 'o': 4x512 cols of w.
    'm': 4x(512 w + 512 m) cols."""
    ent = []
    off = 0
    for kb in range(KBS):
        cls = pat[ub][kb]
        if cls == "z":
            continue
        wdt = 4 * 512 if cls == "o" else 4 * 1024
        ent.append((kb, cls, off, wdt))
        off += wdt
    return ent, off


def _ub_order(pat):
    ksets = []
    for ub in range(UBS):
        ks = [
            kb * KPB + i
            for kb in range(KBS)
            if pat[ub][kb] != "z"
            for i in range(KPB)
        ]
        ksets.append(ks)
    order = sorted(range(UBS), key=lambda ub: -len(ksets[ub]))
    return order, ksets


def _build_module(pat):
    nc = bacc.Bacc("TRN2", target_bir_lowering=False, debug=False)

    ub_order, ksets = _ub_order(pat)
    ub_A = ub_order[0]
    ksA = ksets[ub_A]
    layouts = {ub: _layout(pat, ub) for ub in range(UBS)}

    xp_d = nc.dram_tensor(
        "xp", (P, KC * BC), mybir.dt.bfloat16, kind="ExternalInput"
    )  # packed xT: col k*1024+b = x[b, k*128+p]
    st_d = [
        nc.dram_tensor(
            f"s{ub}", (P, max(layouts[ub][1], 512)), mybir.dt.bfloat16,
            kind="ExternalInput",
        )
        for ub in range(UBS)
    ]
    bp = nc.dram_tensor("bp", (P, UC), mybir.dt.float32, kind="ExternalInput")
    outT = nc.dram_tensor("outT", (UNITS, BC), mybir.dt.float32, kind="ExternalOutput")
    o3 = outT.ap().rearrange("(u p) b -> u p b", p=P)  # [12, 128, 1024]

    with tile.TileContext(nc) as tc:
        with (
            tc.tile_pool(name="cpool", bufs=1) as cpool,
            tc.tile_pool(name="xpool", bufs=1) as xpool,
            tc.tile_pool(name="opool", bufs=1) as opool,
            tc.tile_pool(name="mspool", bufs=8) as mspool,
            tc.tile_pool(name="prpool", bufs=1) as prpool,
            tc.tile_pool(name="otpool", bufs=8) as otpool,
            tc.tile_pool(name="pspool", bufs=8, space="PSUM") as pspool,
        ):
            # ---- PE p-state warm-up: dummy matmuls during the DMA head
            # (PE is otherwise idle until ~12.5us; ramped p-state needs
            # >3us of continuous busy). Reads a memset tile, writes a
            # discarded PSUM slot that later chains reset via start=True.
            dum = cpool.tile([P, BT], mybir.dt.bfloat16, name="dum", tag="dum")
            nc.vector.memset(dum[:], 0.0)
            dps = pspool.tile([P, BT], mybir.dt.float32, name="dps", tag="ps")
            for _i in range(24):
                nc.tensor.matmul(
                    dps[:], dum[:, :P], dum[:, :], start=True, stop=True
                )

            # ---- weight/mask streams (scalar HWDGE queue) ----
            otiles = {}  # (ub, kb) -> [128, 2048] tile of w (all-one blocks)
            prods = {}  # (ub, k) -> [128, 512] masked-weight tile
            nstream = [0]

            def emit_stream(ub, eng):
                ents, _ = layouts[ub]
                for kb, cls, off, wdt in ents:
                    if cls == "o":
                        t = otiles[(ub, kb)] = opool.tile(
                            [P, wdt], mybir.dt.bfloat16,
                            name=f"os{ub}_{kb}", tag=f"os{ub}_{kb}",
                        )
                        eng.dma_start(t[:], st_d[ub].ap()[:, off : off + wdt])
                        nstream[0] += KPB
                    else:
                        # one tile per 2 k-chunks: [w_k|m_k|w_k1|m_k1] --
                        # halves the serialized dma_start trigger count
                        kstep = 2
                        for ki in range(0, KPB, kstep):
                            k = kb * KPB + ki
                            ko = off + ki * 1024
                            t = mspool.tile(
                                [P, kstep * 1024], mybir.dt.bfloat16,
                                name=f"ms{ub}_{k}", tag="ms",
                            )
                            nsp = 2 if nstream[0] < 4 else 1
                            step = kstep * 1024 // nsp
                            for s in range(nsp):
                                eng.dma_start(
                                    t[:, s * step : (s + 1) * step],
                                    st_d[ub].ap()[:, ko + s * step : ko + (s + 1) * step],
                                )
                            for kk in range(kstep):
                                pr = prods[(ub, k + kk)] = prpool.tile(
                                    [P, BLK], mybir.dt.bfloat16,
                                    name=f"pr{ub}_{k + kk}", tag=f"pr{ub}_{k + kk}",
                                )
                                nc.vector.tensor_mul(
                                    pr[:],
                                    t[:, kk * 1024 : kk * 1024 + 512],
                                    t[:, kk * 1024 + 512 : (kk + 1) * 1024],
                                )
                            nstream[0] += kstep

            def wslice(u, k):
                ub, j = divmod(u, UPB)
                kb, ki = divmod(k, KPB)
                if pat[ub][kb] == "o":
                    return otiles[(ub, kb)][:, ki * 512 + j * P : ki * 512 + (j + 1) * P]
                return prods[(ub, k)][:, j * P : (j + 1) * P]

            # ---- x loads (sync HWDGE queue), consumption order ----
            xt = []
            for k in range(KC):
                t = xpool.tile([P, BC], mybir.dt.bfloat16, name=f"x{k}", tag=f"x{k}")
                nsp = 2 if k < 2 else 1
                step = BC // nsp
                for s in range(nsp):
                    nc.sync.dma_start(
                        t[:, s * step : (s + 1) * step],
                        xp_d.ap()[:, k * BC + s * step : k * BC + (s + 1) * step],
                    )
                xt.append(t)

            btile = cpool.tile([P, UC], mybir.dt.float32, name="btile", tag="btile")
            nc.sync.dma_start(btile[:], bp.ap())

            def xslice(k, b):
                return xt[k][:, b * BT : (b + 1) * BT]

            # stream emission: phase A's on the scalar queue (issues from
            # t~7), the other unit-blocks' on the sync queue right after x
            # (issue ~24+, arrive ~40) so their DVE premultiplies -- and
            # therefore phase A's drains behind them in the in-order DVE
            # queue -- are never late; late drains stall every matmul
            # emitted after them
            emit_stream(ub_A, nc.scalar)
            if len(ub_order) > 1:
                emit_stream(ub_order[1], nc.scalar)

            def chain_mm(u, b, ps, k, ks):
                nc.tensor.matmul(
                    ps[:],
                    wslice(u, k),
                    xslice(k, b),
                    start=(k == ks[0]),
                    stop=(k == ks[-1]),
                )

            ndrained = [0]

            def drain(u, b, ps):
                ot = otpool.tile([P, BT], mybir.dt.float32, name=f"ot{u}_{b}", tag="ot")
                if ndrained[0] < 8:
                    # phase A: ScalarE (idle once stream triggers are out);
                    # keeps drains off the in-order DVE mul queue
                    nc.scalar.add(ot[:], ps[:], btile[:, u : u + 1])
                elif ndrained[0] == 2 * UC - 1:
                    # final drain: two half-drains so both half-stores
                    # launch on parallel queues immediately
                    H = BT // 2
                    dst = o3[u][:, b * BT : (b + 1) * BT]
                    for h in range(2):
                        nc.vector.tensor_add(
                            ot[:, h * H : (h + 1) * H],
                            ps[:, h * H : (h + 1) * H],
                            btile[:, u : u + 1].to_broadcast((P, H)),
                        )
                        q = H // 2
                        for s in range(2):
                            eng = nc.sync if s == 0 else nc.scalar
                            lo = h * H + s * q
                            eng.dma_start(dst[:, lo : lo + q], ot[:, lo : lo + q])
                    ndrained[0] += 1
                    return
                else:
                    nc.vector.tensor_add(
                        ot[:], ps[:], btile[:, u : u + 1].to_broadcast((P, BT))
                    )
                ndrained[0] += 1
                dst = o3[u][:, b * BT : (b + 1) * BT]
                # progressively finer splits near the end, alternating HWDGE
                # queues, so the final transfers don't serialize the tail
                if ndrained[0] >= 2 * UC - 1:
                    nsp = 4
                elif ndrained[0] >= 2 * UC - 3:
                    nsp = 2
                else:
                    nsp = 1
                step = BT // nsp
                for s in range(nsp):
                    eng = nc.scalar if (nsp > 1 and s % 2) else nc.sync
                    eng.dma_start(
                        dst[:, s * step : (s + 1) * step],
                        ot[:, s * step : (s + 1) * step],
                    )

            # ---- phase A: k-major over 4 u-chunks (8 banks), u-major tail ----
            uA = [ub_A * UPB + j for j in range(UPB)]
            psA = {}
            for u in uA:
                for b in range(NB):
                    psA[(u, b)] = pspool.tile(
                        [P, BT], mybir.dt.float32, name=f"ps{u}_{b}", tag="ps"
                    )
            split = max(0, len(ksA) - 8)
            for k in ksA[:split]:
                for u in uA:
                    for b in range(NB):
                        chain_mm(u, b, psA[(u, b)], k, ksA)
            for u in uA:
                for k in ksA[split:]:
                    for b in range(NB):
                        chain_mm(u, b, psA[(u, b)], k, ksA)
                for b in range(NB):
                    drain(u, b, psA[(u, b)])

            if len(ub_order) > 2:
                emit_stream(ub_order[2], nc.scalar)

            # ---- phase B: remaining unit-chunks u-major ----
            for ub in ub_order[1:]:
                ks = ksets[ub]
                for j in range(UPB):
                    u = ub * UPB + j
                    pss = [
                        pspool.tile(
                            [P, BT], mybir.dt.float32, name=f"ps{u}_{b}", tag="ps"
                        )
                        for b in range(NB)
                    ]
                    if ub == ub_order[-1] and j == UPB - 1:
                        # b-serial: only the final b-tile's drain + store
                        # remains after the very last matmul
                        for b in range(NB):
                            for k in ks:
                                chain_mm(u, b, pss[b], k, ks)
                            drain(u, b, pss[b])
                    else:
                        for k in ks:
                            for b in range(NB):
                                chain_mm(u, b, pss[b], k, ks)
                        for b in range(NB):
                            drain(u, b, pss[b])

    nc.compile()
    return nc


def get_module(pat):
    if pat not in _MODULES:
        _MODULES[pat] = _build_module(pat)
    return _MODULES[pat]


def make_in_maps(pat, x, w, b, mask):
    x16 = x.astype(BF16)
    w16 = w.astype(BF16)  # (3072, 1536)
    m16 = np.ascontiguousarray(mask.T).astype(BF16)  # (3072, 1536)
    shared = {"bp": np.ascontiguousarray(b.astype(np.float32).reshape(UC, P).T)}
    for ub in range(UBS):
        wk = np.ascontiguousarray(w16[:, ub * BLK : (ub + 1) * BLK]).reshape(
            KC, P, BLK
        )
        mk = np.ascontiguousarray(m16[:, ub * BLK : (ub + 1) * BLK]).reshape(
            KC, P, BLK
        )
        ents, total = _layout(pat, ub)
        stream = np.zeros((P, max(total, 512)), dtype=BF16)
        for kb, cls, off, wdt in ents:
            for ki in range(KPB):
                k = kb * KPB + ki
                if cls == "o":
                    stream[:, off + ki * 512 : off + (ki + 1) * 512] = wk[k]
                else:
                    stream[:, off + ki * 1024 : off + ki * 1024 + 512] = wk[k]
                    stream[:, off + ki * 1024 + 512 : off + (ki + 1) * 1024] = mk[k]
        shared[f"s{ub}"] = stream
    in_maps = []
    for c in range(N_CORES):
        d = dict(shared)
        xc = np.ascontiguousarray(x16[c * BC : (c + 1) * BC].T)  # (3072, 1024)
        d["xp"] = np.ascontiguousarray(
            xc.reshape(KC, P, BC).transpose(1, 0, 2).reshape(P, KC * BC)
        )
        in_maps.append(d)
    return in_maps


def assemble(results):
    out = np.empty((BATCH, UNITS), dtype=np.float32)
    for c in range(N_CORES):
        out[c * BC : (c + 1) * BC, :] = results[c]["outT"].T
    return out


def kernel(x, w, b, mask, _trace=False, _trace_kwargs=None):
    x = np.asarray(x, dtype=np.float32)
    w = np.asarray(w, dtype=np.float32)
    b = np.asarray(b, dtype=np.float32)
    mask = np.asarray(mask, dtype=np.float32)
    pat = _classify(mask)
    nc = get_module(pat)
    in_maps = make_in_maps(pat, x, w, b, mask)
    res = run_bass_kernel_spmd(
        nc,
        in_maps,
        core_ids=list(range(N_CORES)),
        trace=_trace,
        **(_trace_kwargs or {}),
    )
    out = assemble(res.results)
    if _trace:
        return out, res
    return out
